# revision 24
# baseline (speedup 1.0000x reference)
"""Trainium2 Bass kernel for nn_MultiHeadAttention_24824910971155.

Data-parallel over batch: core b computes batch element b (B=8 == n_cores).

Per-core pipeline:
  1. PE-transpose q,k,v on load (x^T needed as matmul operands).
  2. Projections (float32r matmuls, 1 cyc/row): QT = (q@Wq)^T, KT = (k@Wk)^T
     stored [D, L] in f32r; V = v@Wv stored natural [L, D] in bf16 with a
     ones-column appended per head ("Vplus"). V first, then K, then Q, with
     per-128-row tiles so attention heads unblock as projections land.
  3. Per head: S^T = KT_h^T . QT_h (f32r) -> exp(S^T/8) on ScalarE -> expS^T
     in bf16, one 2MB DMA out per head (the attn output, transposed +
     unnormalized; the host normalizes + transposes).
  4. attn@V (bf16): o^T_h = Vplus_h^T . expS^T accumulated in PSUM; the ones
     column yields softmax row-sums for free. Row-sums DMA'd out; 1/rowsum
     applied to o^T on device (PE-replicate of the rowsum + fast reciprocal).
  5. o^T staged through DRAM (lane re-alignment), read back per 128-row chunk
     + qh^T residual from QT, kt-outer fc matmul (f32r), fused
     relu+residual(q)+rowsum via scalar_tensor_tensor, instance-norm.

Host assembles: attn[b,h,q,k] = f32(expst[b,h,k,q]) / rowsum[b,h,q].
"""

import numpy as np
from contextlib import ExitStack

B, L, D, H = 8, 1024, 1024, 16
DH = D // H          # 64
TEMP = float(DH) ** 0.5  # 8.0
EPS = 1e-6
N_CORES = 8
NT = D // 128        # 8 partition tiles
NC2 = L // 512       # 2 lq chunks

_cache = {}


def _build():
    from concourse import bacc
    import concourse.mybir as mybir
    import concourse.tile as tile
    from concourse.masks import make_identity

    F32 = mybir.dt.float32
    F32R = mybir.dt.float32r
    BF16 = mybir.dt.bfloat16
    AF = mybir.ActivationFunctionType
    ALU = mybir.AluOpType

    nc = bacc.Bacc("TRN2", target_bir_lowering=False, debug=False)

    q_d = nc.dram_tensor("q", [L, D], F32, kind="ExternalInput")
    k_d = nc.dram_tensor("k", [L, D], F32, kind="ExternalInput")
    v_d = nc.dram_tensor("v", [L, D], F32, kind="ExternalInput")
    wq_d = nc.dram_tensor("Wq", [D, D], F32R, kind="ExternalInput")
    wk_d = nc.dram_tensor("Wk", [D, D], F32R, kind="ExternalInput")
    wv_d = nc.dram_tensor("Wv", [D, D], F32R, kind="ExternalInput")
    wfc_d = nc.dram_tensor("Wfc", [D, D], F32R, kind="ExternalInput")

    expst_d = nc.dram_tensor("expst", [H, L, L], BF16, kind="ExternalOutput")
    rowsum_d = nc.dram_tensor("rowsum", [H, L], F32R, kind="ExternalOutput")
    o_d = nc.dram_tensor("o", [L, D], F32, kind="ExternalOutput")

    oT_d = nc.dram_tensor("oT_scratch", [D, L], F32R)  # internal staging

    ts = lambda i, s: slice(i * s, (i + 1) * s)

    with tile.TileContext(nc) as tc, ExitStack() as ctx:
        constp = ctx.enter_context(tc.tile_pool(name="const", bufs=1))
        pers = ctx.enter_context(tc.tile_pool(name="pers", bufs=1))
        # PSUM: "mm" ring [128,1024] (2 banks) x2 shared by transposes,
        # projections, S^T and fc; attn@V accumulators + rowsum-replicate
        # take the other 4 banks.
        mmps = ctx.enter_context(tc.tile_pool(name="mmps", bufs=2, space="PSUM"))
        opsp = ctx.enter_context(tc.tile_pool(name="ops", bufs=2, space="PSUM"))
        rpsp = ctx.enter_context(tc.tile_pool(name="rps", bufs=2, space="PSUM"))

        ident = constp.tile([128, 128], F32)
        make_identity(nc, ident[:])
        ones_grid = constp.tile([128, 128], F32)
        nc.vector.memset(ones_grid[:], 1.0)
        ones_row = constp.tile([65, 64], F32R)
        nc.vector.tensor_copy(ones_row[:], ones_grid[0:65, 0:64])

        QT = [pers.tile([128, L], F32R, tag=f"QT{i}", name=f"QT{i}") for i in range(NT)]
        KT = [pers.tile([128, L], F32R, tag=f"KT{i}", name=f"KT{i}") for i in range(NT)]
        V = [
            pers.tile([128, H * (DH + 1)], BF16, tag=f"V{i}", name=f"V{i}")
            for i in range(NT)
        ]
        for mt in range(NT):
            vh = V[mt][:].rearrange("p (h x) -> p h x", x=DH + 1)
            nc.vector.tensor_copy(vh[:, :, DH], ones_grid[:, 0:H])

        def transpose_into(xT, x_d, pool, kind):
            """PE-transpose DRAM [L, D] tensor into xT [128, NT, L] (bf16)."""
            for lt in range(NT):
                x_nat = pool.tile([128, D], F32, tag="x_nat", name=f"xn_{kind}{lt}")
                nc.sync.dma_start(x_nat[:], x_d.ap()[ts(lt, 128), :])
                for half in range(2):
                    ps_t = mmps.tile([128, 512], F32, tag="mm", name="tps")
                    for j in range(4):
                        kt = 4 * half + j
                        nc.tensor.transpose(
                            ps_t[:, ts(j, 128)], x_nat[:, ts(kt, 128)], ident[:]
                        )
                    nc.vector.tensor_copy(
                        xT[:, ts(half, 4), ts(lt, 128)],
                        ps_t[:].rearrange("p (b x) -> p b x", b=4),
                    )

        # ---- Stage A/B part 1: V projection (whole Wv resident) ----
        with tc.tile_pool(name="abv", bufs=2) as abv:
            with tc.tile_pool(name="abv1", bufs=1) as abv1:
                vT = abv1.tile([128, NT, L], F32R, tag="vT")
                transpose_into(vT, v_d, abv, "v")
                wv_sb = abv1.tile([128, NT, D], F32R, tag="Wv")
                nc.sync.dma_start(
                    wv_sb[:], wv_d.ap().rearrange("(t p) n -> p t n", p=128)
                )
                for mt in range(NT):
                    ps = mmps.tile([128, 1024], F32, tag="mm", name="vps")
                    for kt in range(NT):
                        for c in range(2):
                            nc.tensor.matmul(
                                ps[:, ts(c, 512)],
                                vT[:, kt, ts(mt, 128)],
                                wv_sb[:, kt, ts(c, 512)],
                                start=(kt == 0),
                                stop=(kt == NT - 1),
                            )
                    vh = V[mt][:].rearrange("p (h x) -> p h x", x=DH + 1)
                    for c in range(2):
                        nc.vector.tensor_copy(
                            vh[:, ts(c, 8), 0:DH],
                            ps[:, ts(c, 512)].rearrange("p (h x) -> p h x", x=DH),
                        )

        def project(dst_tiles, xT, w_d, kind, abkq, nt):
            """One 128-col slice of a q/k projection: dst[nt] = (x @ W)^T rows."""
            w_nt = abkq.tile([128, NT, 128], F32R, tag="w_nt", name=f"w_{kind}{nt}")
            nc.sync.dma_start(
                w_nt[:],
                w_d.ap()[:, ts(nt, 128)].rearrange("(t p) c -> p t c", p=128),
            )
            ps = mmps.tile([128, 1024], F32, tag="mm", name="pps")
            for kt in range(NT):
                for c in range(2):
                    nc.tensor.matmul(
                        ps[:, ts(c, 512)],
                        w_nt[:, kt, :],
                        xT[:, kt, ts(c, 512)],
                        start=(kt == 0),
                        stop=(kt == NT - 1),
                    )
            nc.vector.tensor_copy(dst_tiles[nt][:], ps[:])

        def attend(h, expool, stgp, smallp, oT_dt):
            """One attention head: S^T, exp, DMA out, attn@V, normalize, stage."""
            ht, po = h // 2, (h % 2) * 64
            ex = expool.tile([128, NT, L], BF16, tag="expst", name=f"ex{h}")
            for t in range(NT):
                ps_s = mmps.tile([128, 1024], F32, tag="mm", name="sps")
                for c in range(NC2):
                    nc.tensor.matmul(
                        ps_s[:, ts(c, 512)],
                        KT[ht][po : po + 64, ts(t, 128)],
                        QT[ht][po : po + 64, ts(c, 512)],
                        start=True,
                        stop=True,
                    )
                nc.scalar.activation(ex[:, t, :], ps_s[:], AF.Exp, scale=1.0 / TEMP)
            nc.sync.dma_start(
                expst_d.ap()[h].rearrange("(t p) l -> p t l", p=128), ex[:]
            )

            ps_os = [
                opsp.tile([65, 512], F32, tag="ops", name=f"ops{c}")
                for c in range(NC2)
            ]
            for t in range(NT):
                for c in range(NC2):
                    nc.tensor.matmul(
                        ps_os[c][:],
                        V[t][:, h * (DH + 1) : (h + 1) * (DH + 1)],
                        ex[:, t, ts(c, 512)],
                        start=(t == 0),
                        stop=(t == NT - 1),
                    )
            for c in range(NC2):
                ps_o = ps_os[c]
                # rowsum (f32r) for DMA + replicate
                rsr = smallp.tile([65, 512], F32R, tag="rsr", name="rsr")
                nc.vector.tensor_copy(rsr[64:65, :], ps_o[64:65, :])
                nc.gpsimd.dma_start(
                    rowsum_d.ap()[h : h + 1, ts(c, 512)], rsr[64:65, :]
                )
                # replicate rowsum to 64 partitions via K=1 matmul
                rep = rpsp.tile([64, 512], F32, tag="rps", name="rep")
                nc.tensor.matmul(
                    rep[0:64, :], ones_row[64:65, :], rsr[64:65, :],
                    start=True, stop=True,
                )
                scr = smallp.tile([64, 512], F32, tag="scr", name="scr")
                rec = smallp.tile([64, 512], F32, tag="rec", name="rec")
                nc.vector.reciprocal_approx_accurate(rec[:], rep[0:64, :], scr[:])
                stg = stgp.tile([64, 512], F32R, tag="stg", name="stg")
                nc.vector.tensor_mul(stg[:], ps_o[0:64, :], rec[:])
                nc.sync.dma_start(
                    oT_dt[h * DH : (h + 1) * DH, ts(c, 512)], stg[:]
                )

        # ---- K projection, then Q projection, then attention ----
        with (
            tc.tile_pool(name="dramp", bufs=1, space="DRAM") as drp,
            tc.tile_pool(name="expool", bufs=3) as expool,
            tc.tile_pool(name="stgp", bufs=4) as stgp,
            tc.tile_pool(name="smalls", bufs=2) as smallp,
        ):
            oT_dt = drp.tile([D, L], F32R, tag="oTd")
            with tc.tile_pool(name="abkq", bufs=2) as abkq:
                with tc.tile_pool(name="xTk", bufs=1) as xtpk:
                    xTk = xtpk.tile([128, NT, L], F32R, tag="xT", name="xT_k")
                    transpose_into(xTk, k_d, abkq, "k")
                    for nt in range(NT):
                        project(KT, xTk, wk_d, "k", abkq, nt)
                with tc.tile_pool(name="xTq", bufs=1) as xtpq:
                    xTq = xtpq.tile([128, NT, L], F32R, tag="xT", name="xT_q")
                    transpose_into(xTq, q_d, abkq, "q")
                    for i in range(NT):
                        project(QT, xTq, wq_d, "q", abkq, i)
            for h in range(H):
                attend(h, expool, stgp, smallp, oT_dt)

        # ---- Stage D: o^T + qh^T residual, fc, relu+res+instnorm ----
        with (
            tc.tile_pool(name="dp", bufs=1) as dp,
            tc.tile_pool(name="fcs", bufs=8) as fcsp,
            tc.tile_pool(name="dsm", bufs=2) as dsm,
        ):
            wfc = dp.tile([128, NT, D], F32R, tag="Wfc")
            nc.sync.dma_start(wfc[:], wfc_d.ap().rearrange("(t p) n -> p t n", p=128))
            oT = [dp.tile([128, L], F32R, tag=f"oT{i}", name=f"oT{i}") for i in range(NT)]
            for kt in range(NT):
                nc.sync.dma_start(oT[kt][:], oT_dt[ts(kt, 128), :])
                nc.vector.tensor_add(oT[kt][:], oT[kt][:], QT[kt][:])

            sums = constp.tile([128, NT], F32)
            ssq = constp.tile([128, NT], F32)

            fc_tiles = []
            for grp in range(4):
                ps_m = [
                    mmps.tile([128, 1024], F32, tag="mm", name=f"fps{grp}_{i}")
                    for i in range(2)
                ]
                for kt in range(NT):
                    for i in range(2):
                        m = grp * 2 + i
                        for c in range(2):
                            nc.tensor.matmul(
                                ps_m[i][:, ts(c, 512)],
                                oT[kt][:, ts(m, 128)],
                                wfc[:, kt, ts(c, 512)],
                                start=(kt == 0),
                                stop=(kt == NT - 1),
                            )
                for i in range(2):
                    m = grp * 2 + i
                    q_nat = dsm.tile([128, D], F32, tag="q_res")
                    nc.sync.dma_start(q_nat[:], q_d.ap()[ts(m, 128), :])
                    fc_sb = fcsp.tile([128, D], F32, tag="fc")
                    nc.vector.scalar_tensor_tensor(
                        fc_sb[:], ps_m[i][:], 0.0, q_nat[:], ALU.max, ALU.add,
                        accum_out=sums[:, m : m + 1],
                    )
                    sq_scr = dsm.tile([128, D], F32, tag="sq")
                    nc.scalar.activation(
                        sq_scr[:], fc_sb[:], AF.Square, accum_out=ssq[:, m : m + 1]
                    )
                    fc_tiles.append(fc_sb)

            # instance-norm stats over D (batched across the 8 row-tiles)
            def stat(tag):
                return constp.tile([128, NT], F32, tag=tag, name=tag)

            mean, ms, mean2, var = stat("mean"), stat("ms"), stat("mean2"), stat("var")
            rcp, rstd, y2, f_t = stat("rcp"), stat("rstd"), stat("y2"), stat("f_t")
            scr8 = stat("scr8")
            nc.vector.tensor_scalar(mean[:], sums[:], 1.0 / D, None, ALU.mult)
            nc.vector.tensor_scalar(ms[:], ssq[:], 1.0 / D, None, ALU.mult)
            nc.vector.tensor_mul(mean2[:], mean[:], mean[:])
            nc.vector.tensor_sub(var[:], ms[:], mean2[:])
            nc.vector.tensor_scalar(var[:], var[:], EPS, None, ALU.add)  # var+eps
            nc.vector.reciprocal_approx_accurate(rcp[:], var[:], scr8[:])
            nc.scalar.activation(rstd[:], rcp[:], AF.Sqrt)
            # one Newton step for rstd = 1/sqrt(var+eps)
            nc.vector.tensor_mul(y2[:], rstd[:], rstd[:])
            nc.vector.tensor_mul(y2[:], var[:], y2[:])
            nc.vector.tensor_scalar(f_t[:], y2[:], -0.5, 1.5, ALU.mult, ALU.add)
            nc.vector.tensor_mul(rstd[:], rstd[:], f_t[:])

            for m in range(NT):
                nc.vector.tensor_scalar(
                    fc_tiles[m][:], fc_tiles[m][:],
                    mean[:, m : m + 1], rstd[:, m : m + 1],
                    ALU.subtract, ALU.mult,
                )
                nc.sync.dma_start(o_d.ap()[ts(m, 128), :], fc_tiles[m][:])

    nc.compile()
    return nc


def _get_nc():
    if "nc" not in _cache:
        _cache["nc"] = _build()
    return _cache["nc"]


def run_cores(in_maps, **kw):
    """Run the SPMD kernel on cores 0..7; returns BassKernelResults."""
    from concourse.bass_utils import run_bass_kernel_spmd

    return run_bass_kernel_spmd(_get_nc(), in_maps, list(range(N_CORES)), **kw)


def kernel(q, k, v, Wq, Wk, Wv, Wfc, _trace=False, _results_out=None):
    q = np.ascontiguousarray(np.asarray(q, dtype=np.float32))
    k = np.ascontiguousarray(np.asarray(k, dtype=np.float32))
    v = np.ascontiguousarray(np.asarray(v, dtype=np.float32))
    Wq = np.ascontiguousarray(np.asarray(Wq, dtype=np.float32))
    Wk = np.ascontiguousarray(np.asarray(Wk, dtype=np.float32))
    Wv = np.ascontiguousarray(np.asarray(Wv, dtype=np.float32))
    Wfc = np.ascontiguousarray(np.asarray(Wfc, dtype=np.float32))

    in_maps = [
        {"q": q[b], "k": k[b], "v": v[b], "Wq": Wq, "Wk": Wk, "Wv": Wv, "Wfc": Wfc}
        for b in range(B)
    ]
    res = run_cores(in_maps, trace=_trace)
    if _results_out is not None:
        _results_out.append(res)

    o = np.stack([res.results[b]["o"] for b in range(B)])
    expst = np.stack(
        [np.asarray(res.results[b]["expst"], dtype=np.float32) for b in range(B)]
    )  # [B,H,lk,lq]
    rowsum = np.stack([res.results[b]["rowsum"] for b in range(B)])  # [B,H,lq]
    attn = expst.transpose(0, 1, 3, 2) / rowsum[:, :, :, None]
    return o, attn


# revision 25
# speedup vs baseline: 1.0186x; 1.0186x over previous
"""Trainium2 Bass kernel for nn_MultiHeadAttention_24824910971155.

Data-parallel over batch: core b computes batch element b (B=8 == n_cores).

Per-core pipeline:
  1. PE-transpose q,k,v on load (x^T needed as matmul operands).
  2. Projections (float32r matmuls, 1 cyc/row): QT = (q@Wq)^T, KT = (k@Wk)^T
     stored [D, L] in f32r; V = v@Wv stored natural [L, D] in bf16 with a
     ones-column appended per head ("Vplus"). V first, then K, then Q, with
     per-128-row tiles so attention heads unblock as projections land.
  3. Per head: S^T = KT_h^T . QT_h (f32r) -> exp(S^T/8) on ScalarE -> expS^T
     in bf16, one 2MB DMA out per head (the attn output, transposed +
     unnormalized; the host normalizes + transposes).
  4. attn@V (bf16): o^T_h = Vplus_h^T . expS^T accumulated in PSUM; the ones
     column yields softmax row-sums for free. Row-sums DMA'd out; 1/rowsum
     applied to o^T on device (PE-replicate of the rowsum + fast reciprocal).
  5. o^T staged through DRAM (lane re-alignment), read back per 128-row chunk
     + qh^T residual from QT, kt-outer fc matmul (f32r), fused
     relu+residual(q)+rowsum via scalar_tensor_tensor, instance-norm.

Host assembles: attn[b,h,q,k] = f32(expst[b,h,k,q]) / rowsum[b,h,q].
"""

import numpy as np
from contextlib import ExitStack

B, L, D, H = 8, 1024, 1024, 16
DH = D // H          # 64
TEMP = float(DH) ** 0.5  # 8.0
EPS = 1e-6
N_CORES = 8
NT = D // 128        # 8 partition tiles
NC2 = L // 512       # 2 lq chunks

_cache = {}


def _build():
    from concourse import bacc
    import concourse.mybir as mybir
    import concourse.tile as tile
    from concourse.masks import make_identity

    F32 = mybir.dt.float32
    F32R = mybir.dt.float32r
    BF16 = mybir.dt.bfloat16
    AF = mybir.ActivationFunctionType
    ALU = mybir.AluOpType

    nc = bacc.Bacc("TRN2", target_bir_lowering=False, debug=False)

    q_d = nc.dram_tensor("q", [L, D], F32, kind="ExternalInput")
    k_d = nc.dram_tensor("k", [L, D], F32, kind="ExternalInput")
    v_d = nc.dram_tensor("v", [L, D], F32, kind="ExternalInput")
    wq_d = nc.dram_tensor("Wq", [D, D], F32R, kind="ExternalInput")
    wk_d = nc.dram_tensor("Wk", [D, D], F32R, kind="ExternalInput")
    wv_d = nc.dram_tensor("Wv", [D, D], F32R, kind="ExternalInput")
    wfc_d = nc.dram_tensor("Wfc", [D, D], BF16, kind="ExternalInput")

    expst_d = nc.dram_tensor("expst", [H, L, L], BF16, kind="ExternalOutput")
    rowsum_d = nc.dram_tensor("rowsum", [H, L], F32R, kind="ExternalOutput")
    o_d = nc.dram_tensor("o", [L, D], F32, kind="ExternalOutput")

    oT_d = nc.dram_tensor("oT_scratch", [D, L], F32R)  # internal staging

    ts = lambda i, s: slice(i * s, (i + 1) * s)

    with tile.TileContext(nc) as tc, ExitStack() as ctx:
        constp = ctx.enter_context(tc.tile_pool(name="const", bufs=1))
        pers = ctx.enter_context(tc.tile_pool(name="pers", bufs=1))
        # PSUM: "mm" ring [128,1024] (2 banks) x2 shared by transposes,
        # projections, S^T and fc; attn@V accumulators + rowsum-replicate
        # take the other 4 banks.
        mmps = ctx.enter_context(tc.tile_pool(name="mmps", bufs=2, space="PSUM"))
        opsp = ctx.enter_context(tc.tile_pool(name="ops", bufs=2, space="PSUM"))
        rpsp = ctx.enter_context(tc.tile_pool(name="rps", bufs=2, space="PSUM"))

        ident = constp.tile([128, 128], F32)
        make_identity(nc, ident[:])
        ones_grid = constp.tile([128, 128], F32)
        nc.vector.memset(ones_grid[:], 1.0)
        ones_row = constp.tile([65, 64], F32R)
        nc.vector.tensor_copy(ones_row[:], ones_grid[0:65, 0:64])

        QT = [pers.tile([128, L], F32R, tag=f"QT{i}", name=f"QT{i}") for i in range(NT)]
        KT = [pers.tile([128, L], F32R, tag=f"KT{i}", name=f"KT{i}") for i in range(NT)]
        V = [
            pers.tile([128, H * (DH + 1)], BF16, tag=f"V{i}", name=f"V{i}")
            for i in range(NT)
        ]
        for mt in range(NT):
            vh = V[mt][:].rearrange("p (h x) -> p h x", x=DH + 1)
            nc.vector.tensor_copy(vh[:, :, DH], ones_grid[:, 0:H])

        def transpose_into(xT, x_d, pool, kind):
            """PE-transpose DRAM [L, D] tensor into xT [128, NT, L] (bf16)."""
            for lt in range(NT):
                x_nat = pool.tile([128, D], F32, tag="x_nat", name=f"xn_{kind}{lt}")
                nc.sync.dma_start(x_nat[:], x_d.ap()[ts(lt, 128), :])
                for half in range(2):
                    ps_t = mmps.tile([128, 512], F32, tag="mm", name="tps")
                    for j in range(4):
                        kt = 4 * half + j
                        nc.tensor.transpose(
                            ps_t[:, ts(j, 128)], x_nat[:, ts(kt, 128)], ident[:]
                        )
                    nc.vector.tensor_copy(
                        xT[:, ts(half, 4), ts(lt, 128)],
                        ps_t[:].rearrange("p (b x) -> p b x", b=4),
                    )

        # ---- Stage A/B part 1: V projection (whole Wv resident) ----
        with tc.tile_pool(name="abv", bufs=2) as abv:
            with tc.tile_pool(name="abv1", bufs=1) as abv1:
                vT = abv1.tile([128, NT, L], F32R, tag="vT")
                transpose_into(vT, v_d, abv, "v")
                wv_sb = abv1.tile([128, NT, D], F32R, tag="Wv")
                nc.sync.dma_start(
                    wv_sb[:], wv_d.ap().rearrange("(t p) n -> p t n", p=128)
                )
                for mt in range(NT):
                    ps = mmps.tile([128, 1024], F32, tag="mm", name="vps")
                    for kt in range(NT):
                        for c in range(2):
                            nc.tensor.matmul(
                                ps[:, ts(c, 512)],
                                vT[:, kt, ts(mt, 128)],
                                wv_sb[:, kt, ts(c, 512)],
                                start=(kt == 0),
                                stop=(kt == NT - 1),
                            )
                    vh = V[mt][:].rearrange("p (h x) -> p h x", x=DH + 1)
                    for c in range(2):
                        nc.vector.tensor_copy(
                            vh[:, ts(c, 8), 0:DH],
                            ps[:, ts(c, 512)].rearrange("p (h x) -> p h x", x=DH),
                        )

        def project(dst_tiles, xT, w_d, kind, abkq, nt):
            """One 128-col slice of a q/k projection: dst[nt] = (x @ W)^T rows."""
            w_nt = abkq.tile([128, NT, 128], F32R, tag="w_nt", name=f"w_{kind}{nt}")
            nc.sync.dma_start(
                w_nt[:],
                w_d.ap()[:, ts(nt, 128)].rearrange("(t p) c -> p t c", p=128),
            )
            ps = mmps.tile([128, 1024], F32, tag="mm", name="pps")
            for kt in range(NT):
                for c in range(2):
                    nc.tensor.matmul(
                        ps[:, ts(c, 512)],
                        w_nt[:, kt, :],
                        xT[:, kt, ts(c, 512)],
                        start=(kt == 0),
                        stop=(kt == NT - 1),
                    )
            nc.vector.tensor_copy(dst_tiles[nt][:], ps[:])

        def attend(h, expool, stgp, smallp, oT_dt):
            """One attention head: S^T, exp, DMA out, attn@V, normalize, stage."""
            ht, po = h // 2, (h % 2) * 64
            ex = expool.tile([128, NT, L], BF16, tag="expst", name=f"ex{h}")
            for t in range(NT):
                ps_s = mmps.tile([128, 1024], F32, tag="mm", name="sps")
                for c in range(NC2):
                    nc.tensor.matmul(
                        ps_s[:, ts(c, 512)],
                        KT[ht][po : po + 64, ts(t, 128)],
                        QT[ht][po : po + 64, ts(c, 512)],
                        start=True,
                        stop=True,
                    )
                nc.scalar.activation(ex[:, t, :], ps_s[:], AF.Exp, scale=1.0 / TEMP)
            nc.sync.dma_start(
                expst_d.ap()[h].rearrange("(t p) l -> p t l", p=128), ex[:]
            )

            ps_os = [
                opsp.tile([65, 512], F32, tag="ops", name=f"ops{c}")
                for c in range(NC2)
            ]
            for t in range(NT):
                for c in range(NC2):
                    nc.tensor.matmul(
                        ps_os[c][:],
                        V[t][:, h * (DH + 1) : (h + 1) * (DH + 1)],
                        ex[:, t, ts(c, 512)],
                        start=(t == 0),
                        stop=(t == NT - 1),
                    )
            for c in range(NC2):
                ps_o = ps_os[c]
                # rowsum (f32r) for DMA + replicate
                rsr = smallp.tile([65, 512], F32R, tag="rsr", name="rsr")
                nc.vector.tensor_copy(rsr[64:65, :], ps_o[64:65, :])
                nc.gpsimd.dma_start(
                    rowsum_d.ap()[h : h + 1, ts(c, 512)], rsr[64:65, :]
                )
                # replicate rowsum to 64 partitions via K=1 matmul
                rep = rpsp.tile([64, 512], F32, tag="rps", name="rep")
                nc.tensor.matmul(
                    rep[0:64, :], ones_row[64:65, :], rsr[64:65, :],
                    start=True, stop=True,
                )
                scr = smallp.tile([64, 512], F32, tag="scr", name="scr")
                rec = smallp.tile([64, 512], F32, tag="rec", name="rec")
                nc.vector.reciprocal_approx_accurate(rec[:], rep[0:64, :], scr[:])
                stg = stgp.tile([64, 512], BF16, tag="stg", name="stg")
                nc.vector.tensor_mul(stg[:], ps_o[0:64, :], rec[:])
                nc.sync.dma_start(
                    oT_dt[h * DH : (h + 1) * DH, ts(c, 512)], stg[:]
                )

        # ---- K projection, then Q projection, then attention ----
        with (
            tc.tile_pool(name="dramp", bufs=1, space="DRAM") as drp,
            tc.tile_pool(name="expool", bufs=3) as expool,
            tc.tile_pool(name="stgp", bufs=4) as stgp,
            tc.tile_pool(name="smalls", bufs=2) as smallp,
        ):
            oT_dt = drp.tile([D, L], BF16, tag="oTd")
            with tc.tile_pool(name="abkq", bufs=2) as abkq:
                with tc.tile_pool(name="xTk", bufs=1) as xtpk:
                    xTk = xtpk.tile([128, NT, L], F32R, tag="xT", name="xT_k")
                    transpose_into(xTk, k_d, abkq, "k")
                    for nt in range(NT):
                        project(KT, xTk, wk_d, "k", abkq, nt)
                with tc.tile_pool(name="xTq", bufs=1) as xtpq:
                    xTq = xtpq.tile([128, NT, L], F32R, tag="xT", name="xT_q")
                    transpose_into(xTq, q_d, abkq, "q")
                    for i in range(NT):
                        project(QT, xTq, wq_d, "q", abkq, i)
            for h in range(H):
                attend(h, expool, stgp, smallp, oT_dt)

        # ---- Stage D: o^T + qh^T residual, fc, relu+res+instnorm ----
        with (
            tc.tile_pool(name="dp", bufs=1) as dp,
            tc.tile_pool(name="fcs", bufs=8) as fcsp,
            tc.tile_pool(name="dsm", bufs=2) as dsm,
        ):
            wfc = dp.tile([128, NT, D], BF16, tag="Wfc")
            nc.sync.dma_start(wfc[:], wfc_d.ap().rearrange("(t p) n -> p t n", p=128))
            oT = [dp.tile([128, L], BF16, tag=f"oT{i}", name=f"oT{i}") for i in range(NT)]
            for kt in range(NT):
                nc.sync.dma_start(oT[kt][:], oT_dt[ts(kt, 128), :])
                nc.vector.tensor_add(oT[kt][:], oT[kt][:], QT[kt][:])

            sums = constp.tile([128, NT], F32)
            ssq = constp.tile([128, NT], F32)

            fc_tiles = []
            for grp in range(4):
                ps_m = [
                    mmps.tile([128, 1024], F32, tag="mm", name=f"fps{grp}_{i}")
                    for i in range(2)
                ]
                for kt in range(NT):
                    for i in range(2):
                        m = grp * 2 + i
                        for c in range(2):
                            nc.tensor.matmul(
                                ps_m[i][:, ts(c, 512)],
                                oT[kt][:, ts(m, 128)],
                                wfc[:, kt, ts(c, 512)],
                                start=(kt == 0),
                                stop=(kt == NT - 1),
                            )
                for i in range(2):
                    m = grp * 2 + i
                    q_nat = dsm.tile([128, D], F32, tag="q_res")
                    nc.sync.dma_start(q_nat[:], q_d.ap()[ts(m, 128), :])
                    fc_sb = fcsp.tile([128, D], F32, tag="fc")
                    nc.vector.scalar_tensor_tensor(
                        fc_sb[:], ps_m[i][:], 0.0, q_nat[:], ALU.max, ALU.add,
                        accum_out=sums[:, m : m + 1],
                    )
                    sq_scr = dsm.tile([128, D], F32, tag="sq")
                    nc.scalar.activation(
                        sq_scr[:], fc_sb[:], AF.Square, accum_out=ssq[:, m : m + 1]
                    )
                    fc_tiles.append(fc_sb)

            # instance-norm stats over D (batched across the 8 row-tiles)
            def stat(tag):
                return constp.tile([128, NT], F32, tag=tag, name=tag)

            mean, ms, mean2, var = stat("mean"), stat("ms"), stat("mean2"), stat("var")
            rcp, rstd, y2, f_t = stat("rcp"), stat("rstd"), stat("y2"), stat("f_t")
            scr8 = stat("scr8")
            nc.vector.tensor_scalar(mean[:], sums[:], 1.0 / D, None, ALU.mult)
            nc.vector.tensor_scalar(ms[:], ssq[:], 1.0 / D, None, ALU.mult)
            nc.vector.tensor_mul(mean2[:], mean[:], mean[:])
            nc.vector.tensor_sub(var[:], ms[:], mean2[:])
            nc.vector.tensor_scalar(var[:], var[:], EPS, None, ALU.add)  # var+eps
            nc.vector.reciprocal_approx_accurate(rcp[:], var[:], scr8[:])
            nc.scalar.activation(rstd[:], rcp[:], AF.Sqrt)
            # one Newton step for rstd = 1/sqrt(var+eps)
            nc.vector.tensor_mul(y2[:], rstd[:], rstd[:])
            nc.vector.tensor_mul(y2[:], var[:], y2[:])
            nc.vector.tensor_scalar(f_t[:], y2[:], -0.5, 1.5, ALU.mult, ALU.add)
            nc.vector.tensor_mul(rstd[:], rstd[:], f_t[:])

            for m in range(NT):
                nc.vector.tensor_scalar(
                    fc_tiles[m][:], fc_tiles[m][:],
                    mean[:, m : m + 1], rstd[:, m : m + 1],
                    ALU.subtract, ALU.mult,
                )
                nc.sync.dma_start(o_d.ap()[ts(m, 128), :], fc_tiles[m][:])

    nc.compile()
    return nc


def _get_nc():
    if "nc" not in _cache:
        _cache["nc"] = _build()
    return _cache["nc"]


def run_cores(in_maps, **kw):
    """Run the SPMD kernel on cores 0..7; returns BassKernelResults."""
    from concourse.bass_utils import run_bass_kernel_spmd

    return run_bass_kernel_spmd(_get_nc(), in_maps, list(range(N_CORES)), **kw)


def kernel(q, k, v, Wq, Wk, Wv, Wfc, _trace=False, _results_out=None):
    q = np.ascontiguousarray(np.asarray(q, dtype=np.float32))
    k = np.ascontiguousarray(np.asarray(k, dtype=np.float32))
    v = np.ascontiguousarray(np.asarray(v, dtype=np.float32))
    Wq = np.ascontiguousarray(np.asarray(Wq, dtype=np.float32))
    Wk = np.ascontiguousarray(np.asarray(Wk, dtype=np.float32))
    Wv = np.ascontiguousarray(np.asarray(Wv, dtype=np.float32))
    Wfc = np.ascontiguousarray(np.asarray(Wfc, dtype=np.float32))

    import ml_dtypes

    wfc16 = Wfc.astype(ml_dtypes.bfloat16)
    in_maps = [
        {"q": q[b], "k": k[b], "v": v[b], "Wq": Wq, "Wk": Wk, "Wv": Wv,
         "Wfc": wfc16}
        for b in range(B)
    ]
    res = run_cores(in_maps, trace=_trace)
    if _results_out is not None:
        _results_out.append(res)

    o = np.stack([res.results[b]["o"] for b in range(B)])
    expst = np.stack(
        [np.asarray(res.results[b]["expst"], dtype=np.float32) for b in range(B)]
    )  # [B,H,lk,lq]
    rowsum = np.stack([res.results[b]["rowsum"] for b in range(B)])  # [B,H,lq]
    attn = expst.transpose(0, 1, 3, 2) / rowsum[:, :, :, None]
    return o, attn
